# revision 14
# baseline (speedup 1.0000x reference)
"""Cross-attention kernel for TRN2, data-parallel over batch (B=8) on 8 cores.

Reference computation per batch element:
    xt  = proj_in(x)              # [L=4096, E=512], 1x1 conv == matmul
    Q   = xt @ W_q.T + b_q
    K   = ctx @ W_k.T + b_k       # ctx: [S=1024, E]
    V   = ctx @ W_v.T + b_v
    att = softmax(Q @ K.T * scale)
    out = proj_out((att @ V).T)   # [C=512, 64, 64]

Host-side algebraic folds (exact up to fp rounding):
  * scale, W_pi, W_q, W_k fold into G = (scale * W_q @ W_pi).T @ W_k, so
    logits.T = (G.T ctx).T-contract X. (The Q'.b_k rank-1 term is constant
    across keys -> softmax-invariant, dropped; a nonzero bias path
    reappears as per-partition q0 bias on the exp.)
  * W_v and W_po fold:  WV = (W_po @ W_v).T ; b_o = b_po + W_po @ b_v
  * softmax normalization is applied at the very end (divide by Z).

On-device the data-dependent weight products are built ONCE per core in
fp32r (GC = s_g * G.T-contract ctx, VW = s_v * ctx.T-contract WV), then
quantized to fp8-e4m3.  The two big per-chunk GEMMs run as fp8 DoubleRow
matmuls (two 128-deep k-tiles per instruction -> 2x PE rate):
  ST[j,i] = GC8.T-contract X8           (PSUM holds s_g * logits)
  tmp     = exp(ST / s_g)               (Act engine, f32)
  R8      = tmp - 1 -> fp8              (DVE; mean-centering shrinks the
                                         fp8 quantization error ~5x)
  Z       = 1024 + ones8.T-contract R8  (tiny fp8 matmul, PE)
  U[o,i]  = A[o] + VW8.T-contract R8    (fp8 DoubleRow)
  y       = (U + A) * (1 / (s_v * Z))   (DVE, invz broadcast via GpSimd)
A[o] = sum_j VW[j,o] is the exact rank-1 correction for the centering,
computed in f32r as (sum_j ctx) @ WV so no fp8 error touches it.
"""

import numpy as np
import ml_dtypes

import concourse.bass as bass
import concourse.mybir as mybir
import concourse.tile as tile
from concourse import bacc
from concourse.bass_utils import run_bass_kernel_spmd

F32 = mybir.dt.float32
F32R = mybir.dt.float32r
F8 = mybir.dt.float8e4
BF16 = mybir.dt.bfloat16
NP_F8 = ml_dtypes.float8_e4m3
NP_BF16 = ml_dtypes.bfloat16
EXP = mybir.ActivationFunctionType.Exp
DR = mybir.MatmulPerfMode.DoubleRow
ADD = mybir.AluOpType.add
MULT = mybir.AluOpType.mult

C = 512       # in channels
E = 512       # emb dim
L = 4096      # query length (64*64)
S = 1024      # key length (32*32)
LI = 512      # i-chunk (query) tile size
NCHUNK = L // LI
NCORES = 8

S_G = 512.0   # fp8 range scale folded into G (logits path)
S_V = 32.0    # fp8 range scale folded into WV (output path)

TRACE = False           # test harness can flip this before calling kernel()
LAST_RESULTS = None     # stashed BassKernelResults for the test harness

_PROGRAM_CACHE = {}


def _round_tf32(a: np.ndarray) -> np.ndarray:
    """Round fp32 mantissa to 11 explicit bits (round-to-nearest-even),
    zeroing the low 12 bits — the fp32r operand format."""
    a = np.ascontiguousarray(a, dtype=np.float32)
    b = a.view(np.uint32)
    r = (b + np.uint32(0x7FF) + ((b >> np.uint32(12)) & np.uint32(1))) & np.uint32(
        0xFFFFF000
    )
    return r.view(np.float32)


def _build_program(has_q0: bool, has_bo: bool):
    nc = bacc.Bacc(
        "TRN2",
        target_bir_lowering=False,
        debug=False,
        enable_asserts=False,
        num_devices=NCORES,
    )
    x_d = nc.dram_tensor("x", [C, L], F8, kind="ExternalInput").ap()
    ctx_d = nc.dram_tensor("ctx", [E, S], BF16, kind="ExternalInput").ap()
    # gt arrives host-permuted into ct-major blocks: gt_d[p, ct*512+et*128+c']
    # = s_g*G.T[et*128+p, ct*128+c'], so the first GC group (ct=0) only needs
    # the first 256KB block and DMA runs stay 2KB-contiguous.
    gt_d = nc.dram_tensor("gt", [128, 4 * C], BF16, kind="ExternalInput").ap()
    wv_d = nc.dram_tensor("wv", [E, E], BF16, kind="ExternalInput").ap()
    # cst: [:, 0:1024] = 4.0 (Z bias k-tile: sum 256*32*4 = s_v*1024),
    # [:, 1024:1280] = 32.0 (s_v, the VW8 ones-block for the Z column group)
    cst_d = nc.dram_tensor("cst", [128, 1280], F8, kind="ExternalInput").ap()
    ab_d = nc.dram_tensor("ab", [128, 4], F32, kind="ExternalInput").ap()
    q0_d = bo_d = None
    if has_q0:
        q0_d = nc.dram_tensor("q0", [128, 8], F32, kind="ExternalInput").ap()
    if has_bo:
        bo_d = nc.dram_tensor("bo", [128, 4], F32, kind="ExternalInput").ap()
    y_d = nc.dram_tensor("y", [C, L], F32, kind="ExternalOutput").ap()

    with tile.TileContext(nc) as tc:
        from contextlib import ExitStack

        with ExitStack() as ctx:
            cpool = ctx.enter_context(tc.tile_pool(name="consts", bufs=1))
            ps_s = ctx.enter_context(tc.tile_pool(name="ps_s", bufs=4, space="PSUM"))
            ps_u = ctx.enter_context(tc.tile_pool(name="ps_u", bufs=4, space="PSUM"))
            xpool = ctx.enter_context(tc.tile_pool(name="xp", bufs=2))
            tpool = ctx.enter_context(tc.tile_pool(name="tp", bufs=2))
            rpool = ctx.enter_context(tc.tile_pool(name="rp", bufs=2))
            opool = ctx.enter_context(tc.tile_pool(name="op", bufs=2))
            zpool = ctx.enter_context(tc.tile_pool(name="zp", bufs=2))

            # ---- loads in latency-priority order --------------------------
            const8 = cpool.tile([128, 2, LI], F8, name="const8")
            nc.sync.dma_start(
                const8[:],
                cst_d[:, 0:1024].rearrange("p (two m) -> p two m", two=2),
            )
            A_sb = cpool.tile([128, 4], F32, name="A_sb")
            nc.sync.dma_start(A_sb[:], ab_d[:, :])
            # warm the PE p-state while the big input DMAs are in flight
            for _ in range(8):
                wps = ps_s.tile([128, LI], F32, name="wps", tag="s")
                nc.tensor.matmul(
                    wps[:], const8[:, :, 0:128], const8[:],
                    start=True, stop=True, perf_mode=DR,
                )
            # interleave gt chunks with ctx first-halves so the jh=0 GC
            # groups (which need gt[ct] + ctx[*, :LI]) complete early.
            GTS = cpool.tile([128, 4 * C], BF16, name="gstk", tag="gstk")
            CTXT = cpool.tile([128, 4 * S], BF16, name="cstk", tag="cstk")
            for h in range(2):
                nc.sync.dma_start(
                    GTS[:, h * 256:(h + 1) * 256], gt_d[:, h * 256:(h + 1) * 256]
                )
            for tt in range(4):
                for h in range(2):
                    nc.sync.dma_start(
                        CTXT[:, tt * S + h * 256:tt * S + (h + 1) * 256],
                        ctx_d[tt * 128:(tt + 1) * 128, h * 256:(h + 1) * 256],
                    )
            for ctb in range(1, 4):
                nc.sync.dma_start(
                    GTS[:, ctb * 512:(ctb + 1) * 512], gt_d[:, ctb * 512:(ctb + 1) * 512]
                )

            def load_x(ic):
                xt = xpool.tile([128, 4, LI], F8, name="xc", tag="x")
                for h in range(2):
                    nc.sync.dma_start(
                        xt[:, 2 * h:2 * h + 2, :],
                        x_d[2 * h * 128:2 * (h + 1) * 128, bass.ts(ic, LI)]
                        .rearrange("(t p) c -> p t c", p=128),
                    )
                return xt

            X0 = load_x(0)                                        # prefetch chunk 0
            for tt in range(4):
                nc.sync.dma_start(
                    CTXT[:, tt * S + LI:(tt + 1) * S],
                    ctx_d[tt * 128:(tt + 1) * 128, LI:S],
                )
            # WVT: [4*128, E] DRAM -> [128, 4*E] SBUF (et-chunk t at offset t*E)
            WVT = cpool.tile([128, 4 * E], BF16, name="wstk", tag="wstk")
            for tt in range(4):
                nc.sync.dma_start(
                    WVT[:, tt * E:(tt + 1) * E], wv_d[tt * 128:(tt + 1) * 128, :]
                )
            q0_s = bo_s = None
            if has_q0:
                q0_s = cpool.tile([128, 8], F32, name="q0s")
                nc.sync.dma_start(q0_s[:], q0_d[:, :])
            if has_bo:
                bo_s = cpool.tile([128, 4], F32, name="bos")
                nc.sync.dma_start(bo_s[:], bo_d[:, :])

            def ctx_blk(et, jt):            # CTX [e-chunk et, j-tile jt]
                return CTXT[:, et * S + jt * 128: et * S + (jt + 1) * 128]

            # ---- GC8[c, j] = fp8(sum_e s_g*G[c, e] ctx[e, j])  ------------
            # f32r matmuls, PSUM f32, quantize on the DVE copy out.
            # jh-outer: the four jh=0 groups need only the ctx first-halves.
            GC8 = [
                cpool.tile([128, 2, S], F8, name=f"gc8_{cp}", tag=f"gc8_{cp}")
                for cp in range(2)
            ]
            # o-cols 0:512 hold s_v*VW; cols 512:640 hold the constant s_v
            # (the "ones" block whose U output column-group is s_v*sum_j R8,
            # replicated across all 128 partitions)
            VW8 = [
                cpool.tile([128, 2, E + 128], F8, name=f"vw8_{jp}", tag=f"vw8_{jp}")
                for jp in range(4)
            ]
            for jp in range(4):
                nc.sync.dma_start(
                    VW8[jp][:, :, E:E + 128],
                    cst_d[:, 1024:1280].rearrange("p (two m) -> p two m", two=2),
                )
            def emit_st_jt(Xc, r8, jt):
                sps = ps_s.tile([128, LI], F32, name="sps", tag="s")
                for cp in range(2):
                    nc.tensor.matmul(
                        sps[:],
                        GC8[cp][:, :, jt * 128:(jt + 1) * 128],
                        Xc[:, cp * 2:cp * 2 + 2, :],
                        start=(cp == 0),
                        stop=(cp == 1),
                        perf_mode=DR,
                    )
                tmp = tpool.tile([128, LI], F32, name="tmp", tag=f"t{jt % 4}")
                if has_q0:
                    nc.scalar.activation(
                        tmp[:], sps[:], EXP,
                        bias=q0_s[:, jt:jt + 1], scale=1.0 / S_G,
                    )
                else:
                    nc.scalar.activation(tmp[:], sps[:], EXP, scale=1.0 / S_G)
                if jt >= 6:
                    nc.scalar.activation(
                        r8[jt // 2][:, jt % 2, :], tmp[:],
                        mybir.ActivationFunctionType.Copy, bias=-1.0,
                    )
                else:
                    nc.vector.tensor_scalar_add(
                        r8[jt // 2][:, jt % 2, :], tmp[:], -1.0
                    )

            # GC build interleaved with chunk-0 ST: the jt<4 logits only need
            # the jh=0 half of GC8, so they fill the PE while the jh=1 ctx
            # DMAs land.  One filler matmul per GC group keeps the PE p-state
            # ramp alive across short DMA waits.
            X = load_x(1)
            r8 = [
                rpool.tile([128, 2, LI], F8, name="r8", tag=f"r{jp}")
                for jp in range(4)
            ]
            for jh in range(2):
                for ct in range(4):
                    gps = ps_s.tile([128, LI], F32, name="gps", tag="s")
                    for et in range(4):
                        nc.tensor.matmul(
                            gps[:],
                            GTS[:, ct * 512 + et * 128: ct * 512 + (et + 1) * 128],
                            CTXT[:, et * S + jh * LI: et * S + (jh + 1) * LI],
                            start=(et == 0),
                            stop=(et == 3),
                        )
                    nc.scalar.copy(
                        GC8[ct // 2][:, ct % 2, jh * LI:(jh + 1) * LI], gps[:]
                    )
                    if jh == 0:
                        wps = ps_s.tile([128, LI], F32, name="wps", tag="s")
                        nc.tensor.matmul(
                            wps[:], const8[:, :, 0:128], const8[:],
                            start=True, stop=True, perf_mode=DR,
                        )
                for jt in range(jh * 4, jh * 4 + 4):
                    emit_st_jt(X0, r8, jt)
            prev = r8

            # ---- main chunk loop, one-chunk software pipeline -------------
            # PE order per iteration: Z(ic-1), [ST(ic) x2, U(ic-1, ot)] x4.
            # While PE streams ST(ic), Act exps chunk ic and DVE writes
            # R8(ic); Z/U consume the chunk-(ic-1) R8 tiles (long ready).

            # ---- VW8[j, o] = fp8(sum_e ctx[e, j] s_v*WV[e, o]) ------------
            for jt in range(8):
                vps = ps_s.tile([128, E], F32, name="vps", tag="s")
                for et in range(4):
                    nc.tensor.matmul(
                        vps[:],
                        ctx_blk(et, jt),
                        WVT[:, bass.ts(et, E)],
                        start=(et == 0),
                        stop=(et == 3),
                    )
                nc.scalar.copy(VW8[jt // 2][:, jt % 2, 0:E], vps[:])

            def emit_z(r8):
                # Z column-group: U-GEMM over the VW8 ones-block (value s_v)
                # plus a constant k-tile contributing s_v*1024, so the PSUM
                # holds s_v*Z[i] replicated on every partition.
                zps = ps_u.tile([128, LI], F32, name="zps", tag="u")
                nc.tensor.matmul(
                    zps[:], VW8[0][:, :, E:E + 128], const8[:],
                    start=True, stop=False, perf_mode=DR,
                )
                for jp in range(4):
                    nc.tensor.matmul(
                        zps[:], VW8[jp][:, :, E:E + 128], r8[jp][:],
                        start=False, stop=(jp == 3), perf_mode=DR,
                    )
                invz_rep = zpool.tile([128, LI], F32, name="invzr", tag="invzr")
                nc.vector.reciprocal_approx_fast(out=invz_rep[:], in_=zps[:])
                return invz_rep

            def emit_u(r8, invz_rep, ic_out):
                isl = bass.ts(ic_out, LI)
                for ot in range(4):
                    ups = ps_u.tile([128, LI], F32, name="ups", tag="u")
                    for jp in range(4):
                        nc.tensor.matmul(
                            ups[:],
                            VW8[jp][:, :, ot * 128:(ot + 1) * 128],
                            r8[jp][:],
                            start=(jp == 0), stop=(jp == 3), perf_mode=DR,
                        )
                    o = opool.tile([128, LI], F32, name="ot", tag=f"o{ot}")
                    # o = (ups + A[o]) * invz ; optional +bo
                    nc.vector.scalar_tensor_tensor(
                        out=o[:], in0=ups[:], scalar=A_sb[:, ot:ot + 1],
                        in1=invz_rep[:], op0=ADD, op1=MULT,
                    )
                    if has_bo:
                        nc.vector.tensor_scalar_add(o[:], o[:], bo_s[:, ot:ot + 1])
                    for h in range(2):
                        nc.sync.dma_start(
                            y_d[ot * 128 + h * 64:ot * 128 + (h + 1) * 64, isl],
                            o[h * 64:(h + 1) * 64, :],
                        )

            for ic in range(1, NCHUNK):
                Xc = X
                if ic + 1 < NCHUNK:
                    X = load_x(ic + 1)      # prefetch next chunk
                invz_rep = emit_z(prev)
                r8 = [
                    rpool.tile([128, 2, LI], F8, name="r8", tag=f"r{jp}")
                    for jp in range(4)
                ]
                for jt in range(8):
                    sps = ps_s.tile([128, LI], F32, name="sps", tag="s")
                    for cp in range(2):
                        nc.tensor.matmul(
                            sps[:],
                            GC8[cp][:, :, jt * 128:(jt + 1) * 128],
                            Xc[:, cp * 2:cp * 2 + 2, :],
                            start=(cp == 0),
                            stop=(cp == 1),
                            perf_mode=DR,
                        )
                    tmp = tpool.tile([128, LI], F32, name="tmp", tag=f"t{jt % 4}")
                    if has_q0:
                        nc.scalar.activation(
                            tmp[:], sps[:], EXP,
                            bias=q0_s[:, jt:jt + 1], scale=1.0 / S_G,
                        )
                    else:
                        nc.scalar.activation(tmp[:], sps[:], EXP, scale=1.0 / S_G)
                    if jt >= 6:
                        nc.scalar.activation(
                            r8[jt // 2][:, jt % 2, :], tmp[:],
                            mybir.ActivationFunctionType.Copy, bias=-1.0,
                        )
                    else:
                        nc.vector.tensor_scalar_add(
                            r8[jt // 2][:, jt % 2, :], tmp[:], -1.0
                        )
                    # interleave U(ic-1) work between ST groups to keep the
                    # PE busy while Act/DVE catch up on chunk ic's exp chain
                    if jt % 2 == 1:
                        isl_prev = bass.ts(ic - 1, LI)
                        ot = jt // 2
                        ups = ps_u.tile([128, LI], F32, name="ups", tag="u")
                        for jp in range(4):
                            nc.tensor.matmul(
                                ups[:],
                                VW8[jp][:, :, ot * 128:(ot + 1) * 128],
                                prev[jp][:],
                                start=(jp == 0), stop=(jp == 3), perf_mode=DR,
                            )
                        o = opool.tile([128, LI], F32, name="ot", tag=f"o{ot}")
                        nc.vector.scalar_tensor_tensor(
                            out=o[:], in0=ups[:], scalar=A_sb[:, ot:ot + 1],
                            in1=invz_rep[:], op0=ADD, op1=MULT,
                        )
                        if has_bo:
                            nc.vector.tensor_scalar_add(o[:], o[:], bo_s[:, ot:ot + 1])
                        for h in range(2):
                            nc.sync.dma_start(
                                y_d[ot * 128 + h * 64:ot * 128 + (h + 1) * 64,
                                    isl_prev],
                                o[h * 64:(h + 1) * 64, :],
                            )
                prev = r8
            # flush final chunk
            invz_rep = emit_z(prev)
            emit_u(prev, invz_rep, NCHUNK - 1)

    nc.compile()
    return nc


def kernel(**inputs) -> np.ndarray:
    global LAST_RESULTS
    x = np.asarray(inputs["x"], dtype=np.float32)
    context = np.asarray(inputs["context"], dtype=np.float32)
    W_pi = np.asarray(inputs["W_pi"], dtype=np.float64)
    b_pi = np.asarray(inputs["b_pi"], dtype=np.float64)
    W_q = np.asarray(inputs["W_q"], dtype=np.float64)
    b_q = np.asarray(inputs["b_q"], dtype=np.float64)
    W_k = np.asarray(inputs["W_k"], dtype=np.float64)
    W_v = np.asarray(inputs["W_v"], dtype=np.float64)
    b_v = np.asarray(inputs["b_v"], dtype=np.float64)
    W_po = np.asarray(inputs["W_po"], dtype=np.float64)
    b_po = np.asarray(inputs["b_po"], dtype=np.float64)

    scale = float(E) ** -0.5
    Wqpi = scale * (W_q @ W_pi)                            # [dq, c]
    G = (Wqpi.T @ W_k)                                     # [c, e]
    GT = (S_G * np.ascontiguousarray(G.T)).astype(NP_BF16)  # [e, c]
    # ct-major block permutation: A[p, ct*512+et*128+c'] = GT[et*128+p, ct*128+c']
    GT = np.ascontiguousarray(
        GT.reshape(4, 128, 4, 128).transpose(1, 2, 0, 3).reshape(128, 4 * C)
    )
    b_row = scale * (W_q @ b_pi + b_q)
    q0_e = (W_k.T @ b_row).astype(np.float64)              # [e]
    WV = (S_V * (W_po @ W_v).T).astype(NP_BF16)    # [e, o]
    b_o = (b_po + W_po @ b_v).astype(np.float32)           # [o]

    has_q0 = bool(np.any(q0_e))
    has_bo = bool(np.any(b_o))
    key = (has_q0, has_bo)
    if key not in _PROGRAM_CACHE:
        _PROGRAM_CACHE[key] = _build_program(has_q0, has_bo)
    nc = _PROGRAM_CACHE[key]

    cst = np.empty((128, 1280), dtype=NP_F8)
    cst[:, 0:1024] = NP_F8(4.0)
    cst[:, 1024:1280] = NP_F8(S_V)
    in_maps = []
    for c in range(NCORES):
        ctx_mat = context[c].reshape(E, S)
        cbar = ctx_mat.astype(np.float64).sum(axis=1)
        ab = (_round_tf32(cbar.astype(np.float32)).astype(np.float64)
              @ WV.astype(np.float64)).astype(np.float32)
        m = {
            "x": x[c].reshape(C, L).astype(NP_F8),
            "ctx": ctx_mat.astype(NP_BF16),
            "gt": GT,
            "wv": WV,
            "cst": cst,
            "ab": np.ascontiguousarray(ab.reshape(4, 128).T),
        }
        if has_q0:
            # logits bias per key j: q0_e . ctx[:, j]  -> [S] -> [128, 8]
            q0j = (q0_e @ ctx_mat.astype(np.float64)).astype(np.float32)
            m["q0"] = np.ascontiguousarray(q0j.reshape(8, 128).T)
        if has_bo:
            m["bo"] = np.ascontiguousarray(b_o.reshape(4, 128).T)
        in_maps.append(m)

    res = run_bass_kernel_spmd(nc, in_maps, core_ids=list(range(NCORES)), trace=TRACE)
    LAST_RESULTS = res
    y = np.stack([res.results[c]["y"] for c in range(NCORES)], axis=0)
    return np.ascontiguousarray(y.reshape(NCORES, C, 64, 64).astype(np.float32))
